# revision 3
# baseline (speedup 1.0000x reference)
"""GraphSAGE mean-aggregator encoder on Trainium2, 8-core SPMD — v2.

Strategy (vs v1's 176 per-tile indirect DMAs at ~1.4us each):
two-hop gather built on the SWDGE `dma_gather` ucode (mlp library), which
moves thousands of rows per Pool-engine instruction (994ns fixed +
0.34ns/desc) instead of 128.

Per core (2048 nodes, 22528 feature-row refs), in 4 rounds of 512 nodes:
  hop-1: the round's 5632 row refs are host-sorted by 32768-row chunk of the
    bf16 feature table so indices fit dma_gather's int16 range. 7 chunked
    dma_gathers (static capacity 6x1088+256, padded with dup indices) land
    rows at host-known positions of a [128, 53*512] SBUF buffer
    (row i -> partition i%128, rank i//128). Double-buffered across rounds.
  hop-2: per 128-node tile, one SBUF-source transpose dma_gather pulls the
    tile's 1408 rows in node order (idx = host-computed hop-1 position),
    emitting feature-major columns [128p, 4 chunks, 1408 cols] that feed the
    PE directly — no PE transposes, no reassembly DVE work.
  compute: 9 strided DVE adds reduce the 10 neighbor columns per node
    (1/S pre-folded into W's neighbor half), 8 bf16 matmuls accumulate
    W_self/W_neigh chunks into PSUM, ACT relu -> [128, 2048] fp32 out,
    single 1MB store at the end.

Everything data-dependent lives in runtime index tensors; the compiled
program is static. bf16 features/W keep rel err ~3e-3 (budget 2e-2).
"""

import numpy as np
import ml_dtypes
from contextlib import ExitStack

import concourse.bass as bass
import concourse.mybir as mybir
import concourse.tile as tile
from concourse import bacc
from concourse.bass_utils import run_bass_kernel_spmd

NCORES = 8
B = 16384
BC = B // NCORES      # 2048 nodes per core
S = 10                # neighbor samples
J = S + 1             # rows per node (self + neighbors)
F = 512               # feature dim
E = 128               # embed dim
NNODES = 200000
P = 128

CH = 1 << 15          # feature-table chunk rows (int16 index range)
NCH = 7               # ceil(200000/32768)
CHUNK_ROWS = [min(NNODES - c * CH, CH) for c in range(NCH)]
ROUNDS = 4
NPR = BC // ROUNDS    # 512 nodes per round
SLOTS_R = NPR * J     # 5632 row refs per round
TPR = NPR // P        # 4 tiles per round
TILES = BC // P       # 16
COLS = P * J          # 1408 hop-2 columns per tile
KCH = F // P          # 4 feature chunks

# static per-chunk hop-1 capacities, multiples of 128 (observed max over
# core-rounds for the reference input: 996 / 135; margins absorb input drift)
CAPS = [1152] * 6 + [256]
CAPSUM = sum(CAPS)    # 6784 slots per round
RANKS1 = CAPSUM // P  # 53
BASE = np.concatenate([[0], np.cumsum(CAPS)]).astype(np.int64)
W1 = CAPSUM // 16     # idx1 columns per round

_CACHE = {}


def build_nc():
    nc = bacc.Bacc(
        "TRN2",
        target_bir_lowering=False,
        debug=False,
        num_devices=NCORES,
    )

    feat = nc.dram_tensor("feat", [NNODES, F], mybir.dt.bfloat16, kind="ExternalInput").ap()
    idx1 = nc.dram_tensor("idx1", [P, ROUNDS * W1], mybir.dt.int16, kind="ExternalInput").ap()
    idx2 = nc.dram_tensor("idx2", [P, TILES * (COLS // 16)], mybir.dt.int16, kind="ExternalInput").ap()
    # host-preprocessed: W^T with the neighbor half pre-scaled by 1/S -> [2F, E] bf16
    wt = nc.dram_tensor("wt", [2 * F, E], mybir.dt.bfloat16, kind="ExternalInput").ap()
    out = nc.dram_tensor("out", [E, BC], mybir.dt.float32, kind="ExternalOutput").ap()

    with tile.TileContext(nc) as tc, ExitStack() as ctx:
        consts = ctx.enter_context(tc.tile_pool(name="consts", bufs=1))
        h1pool = ctx.enter_context(tc.tile_pool(name="h1pool", bufs=2))
        h2pool = ctx.enter_context(tc.tile_pool(name="h2pool", bufs=3))
        nspool = ctx.enter_context(tc.tile_pool(name="nspool", bufs=3))
        psum_o = ctx.enter_context(tc.tile_pool(name="psum_o", bufs=4, space="PSUM"))

        idx1_sb = consts.tile([P, ROUNDS * W1], mybir.dt.int16)
        nc.sync.dma_start(out=idx1_sb[:], in_=idx1[:])
        idx2_sb = consts.tile([P, TILES * (COLS // 16)], mybir.dt.int16)
        nc.sync.dma_start(out=idx2_sb[:], in_=idx2[:])

        # W^T chunks: wt dram rows (k p) -> sbuf [p, (k e)]; chunk k feeds the
        # matmul whose rhs is feature chunk k (0-3 self, 4-7 neighbor-sum)
        wt_sb = consts.tile([P, 2 * KCH * E], mybir.dt.bfloat16)
        nc.sync.dma_start(
            out=wt_sb[:].rearrange("p (k e) -> p k e", k=2 * KCH),
            in_=wt.rearrange("(k p) e -> p k e", k=2 * KCH),
        )

        out_sb = consts.tile([E, BC], mybir.dt.float32)

        for r in range(ROUNDS):
            h1 = h1pool.tile([P, RANKS1 * F], mybir.dt.bfloat16, tag="h1", name=f"h1_{r}")
            h1r = h1[:].rearrange("p (k f) -> p k f", f=F)
            for c in range(NCH):
                nc.gpsimd.dma_gather(
                    out_ap=h1r[:, BASE[c] // P : BASE[c + 1] // P, :],
                    in_ap=feat[c * CH : c * CH + CHUNK_ROWS[c], :],
                    idxs_ap=idx1_sb[:, r * W1 + BASE[c] // 16 : r * W1 + BASE[c + 1] // 16],
                    num_idxs=CAPS[c],
                    num_idxs_reg=CAPS[c],
                    elem_size=F,
                    single_packet=False,
                )

            for t in range(TPR):
                gt = r * TPR + t
                h2 = h2pool.tile([P, KCH * COLS], mybir.dt.bfloat16, tag="h2", name=f"h2_{gt}")
                nc.gpsimd.dma_gather(
                    out_ap=h2[:].rearrange("p (q n) -> p q n", q=KCH),
                    in_ap=h1[:],
                    idxs_ap=idx2_sb[:, gt * (COLS // 16) : (gt + 1) * (COLS // 16)],
                    num_idxs=COLS,
                    num_idxs_reg=COLS,
                    elem_size=F,
                    transpose=True,
                    sbuf_tokens_per_rank=P,
                    sbuf_free_dim_per_rank=F * 2,
                    single_packet=False,
                )
                # [p, chunk, node, j]: j=0 self, 1..10 neighbors
                h2v = h2[:].rearrange("p (q n j) -> p q n j", q=KCH, j=J)

                ns = nspool.tile([P, KCH * P], mybir.dt.bfloat16, tag="ns", name=f"ns_{gt}")
                nsv = ns[:].rearrange("p (q n) -> p q n", q=KCH)
                nc.vector.tensor_add(out=nsv, in0=h2v[:, :, :, 1], in1=h2v[:, :, :, 2])
                for j in range(3, J):
                    nc.vector.tensor_add(out=nsv, in0=nsv, in1=h2v[:, :, :, j])

                po = psum_o.tile([E, P], mybir.dt.float32)
                for q in range(KCH):
                    nc.tensor.matmul(
                        out=po[:],
                        lhsT=wt_sb[:, q * E : (q + 1) * E],
                        rhs=h2v[:, q, :, 0],
                        start=(q == 0),
                        stop=False,
                    )
                for q in range(KCH):
                    nc.tensor.matmul(
                        out=po[:],
                        lhsT=wt_sb[:, (KCH + q) * E : (KCH + q + 1) * E],
                        rhs=nsv[:, q, :],
                        start=False,
                        stop=(q == KCH - 1),
                    )

                nc.scalar.activation(
                    out=out_sb[:, gt * P : (gt + 1) * P],
                    in_=po[:],
                    func=mybir.ActivationFunctionType.Relu,
                )

        nc.sync.dma_start(out=out[:], in_=out_sb[:])

    nc.compile()
    return nc


def _get_nc():
    if "nc" not in _CACHE:
        _CACHE["nc"] = build_nc()
    return _CACHE["nc"]


def _wrap16(flat):
    """Pack a flat idx list: i at [i%16, i//16], replicated across the 8
    16-partition groups -> [128, len/16] int16."""
    n = flat.shape[0]
    a = flat.reshape(n // 16, 16).T.astype(np.int16)  # [16, n/16]
    return np.tile(a, (8, 1))


def make_in_maps(nodes, neigh_idx, features, weight):
    nodes = np.asarray(nodes, dtype=np.int32)
    neigh_idx = np.asarray(neigh_idx, dtype=np.int32)
    features = np.asarray(features, dtype=np.float32)
    weight = np.asarray(weight, dtype=np.float32)

    feat_bf = np.ascontiguousarray(features.astype(ml_dtypes.bfloat16))

    w = weight.copy()
    w[:, F:] *= 1.0 / S
    wt = np.ascontiguousarray(w.T.astype(ml_dtypes.bfloat16))  # [2F, E]

    gidx = np.concatenate([nodes[:, None], neigh_idx], axis=1)  # [B, J]

    in_maps = []
    for core in range(NCORES):
        g_core = gidx[core * BC : (core + 1) * BC]  # [BC, J]
        idx1_cols = []
        pos_all = np.empty(BC * J, dtype=np.int16)
        for r in range(ROUNDS):
            g = g_core[r * NPR : (r + 1) * NPR].ravel().astype(np.int64)  # [SLOTS_R]
            cid = g >> 15
            order = np.argsort(cid, kind="stable")
            counts = np.bincount(cid, minlength=NCH)
            if (counts > np.array(CAPS)).any():
                raise RuntimeError(f"chunk capacity exceeded: {counts} vs {CAPS}")
            starts = np.concatenate([[0], np.cumsum(counts)])
            # hop-1 sorted, chunk-relative, padded-per-chunk index list
            lst = np.zeros(CAPSUM, dtype=np.int16)
            for c in range(NCH):
                seg = g[order[starts[c] : starts[c + 1]]] - c * CH
                lst[BASE[c] : BASE[c] + counts[c]] = seg.astype(np.int16)
            idx1_cols.append(lst)
            # hop-2 positions: slot order[k] landed at BASE[c] + (k - starts[c])
            pos = np.empty(SLOTS_R, dtype=np.int64)
            pos[order] = np.arange(SLOTS_R) - starts[cid[order]] + BASE[cid[order]]
            pos_all[r * SLOTS_R : (r + 1) * SLOTS_R] = pos.astype(np.int16)

        idx1_arr = np.concatenate([_wrap16(l) for l in idx1_cols], axis=1)
        idx2_arr = np.concatenate(
            [_wrap16(pos_all[t * COLS : (t + 1) * COLS]) for t in range(TILES)], axis=1
        )
        in_maps.append(
            {
                "feat": feat_bf,
                "idx1": np.ascontiguousarray(idx1_arr),
                "idx2": np.ascontiguousarray(idx2_arr),
                "wt": wt,
            }
        )
    return in_maps


def run(nodes, neigh_idx, features, weight, trace=False):
    nc = _get_nc()
    in_maps = make_in_maps(nodes, neigh_idx, features, weight)
    res = run_bass_kernel_spmd(nc, in_maps, list(range(NCORES)), trace=trace)
    full = np.concatenate([res.results[c]["out"] for c in range(NCORES)], axis=1)
    return full, res


def kernel(nodes, neigh_idx, features, weight):
    full, _ = run(nodes, neigh_idx, features, weight, trace=False)
    return full


# revision 6
# speedup vs baseline: 1.6732x; 1.6732x over previous
"""GraphSAGE-style mean-aggregator encoder on Trainium2, 8-core SPMD.

Computation (per the reference):
    neigh = features[neigh_idx].mean(1)         # [B, F]
    self_ = features[nodes]                     # [B, F]
    out   = relu(W @ concat(self_, neigh).T)    # [E, B]

Sharding: data-parallel over the node batch B=16384 -> 2048 nodes/core.
features + (pre-transposed, pre-scaled) weight replicated per core.

Per-core kernel, per 128-node tile (16 tiles):
  - 11 single-index indirect DMAs (one per sample) gather 128 rows each
    into whole [128, 512] tiles. HW constraints found empirically: the
    multi-index offset-AP form is mis-ordered and ~70x slower, the offset
    AP must start at a tile base, and indirect-DMA writes to nonzero SBUF
    offsets fault the exec unit. Gathers pipeline at ~1.4us per 256KB
    (Q7 SWDGE descriptor-emission bound, ~181 GB/s/core).
  - neighbor mean via chained DVE adds (1/10 pre-folded into W's
    neighbor half)
  - PE transposes the 8 [128,128] chunks of [self | neigh_sum] via
    identity matmuls, ACT copies PSUM->SBUF; groups of 4 tiles pack rhs
    to N=512 so each W-chunk LoadStationary amortizes (fp32 PE is 4-pass)
  - ACT relu PSUM -> a persistent [128, 2048] output buffer; single 1MB
    store at the end.

Measured on 8xTRN2 (NTFF profile): 282.6us, rel err 4.3e-07.
"""

import numpy as np
from contextlib import ExitStack

import concourse.bass as bass
import concourse.mybir as mybir
import concourse.tile as tile
from concourse import bacc
from concourse.bass_utils import run_bass_kernel_spmd

NCORES = 8
B = 16384
BC = B // NCORES  # 2048 nodes per core
S = 10            # neighbor samples
J = S + 1         # gathered rows per node (self + neighbors)
F = 512           # feature dim
E = 128           # embed dim
NNODES = 200000
P = 128
TILES = BC // P   # 16
IDXW = 16         # padded width of the packed index rows

_CACHE = {}


def build_nc():
    nc = bacc.Bacc(
        "TRN2",
        target_bir_lowering=False,
        debug=False,
        num_devices=NCORES,
    )

    gidx = nc.dram_tensor("gidx", [BC, IDXW], mybir.dt.int32, kind="ExternalInput").ap()
    features = nc.dram_tensor(
        "features", [NNODES, F], mybir.dt.float32, kind="ExternalInput"
    ).ap()
    # host-preprocessed: W^T with the neighbor half pre-scaled by 1/S -> [2F, E]
    wt = nc.dram_tensor("wt", [2 * F, E], mybir.dt.float32, kind="ExternalInput").ap()
    ident = nc.dram_tensor("ident", [P, P], mybir.dt.float32, kind="ExternalInput").ap()
    out = nc.dram_tensor("out", [E, BC], mybir.dt.float32, kind="ExternalOutput").ap()

    KCHUNKS = 2 * F // P  # 8

    with tile.TileContext(nc) as tc, ExitStack() as ctx:
        consts = ctx.enter_context(tc.tile_pool(name="consts", bufs=1))
        stpool = ctx.enter_context(tc.tile_pool(name="stpool", bufs=1))
        gpool = ctx.enter_context(tc.tile_pool(name="gpool", bufs=4))
        spool = ctx.enter_context(tc.tile_pool(name="spool", bufs=3))
        ctpool = ctx.enter_context(tc.tile_pool(name="ctpool", bufs=12))
        psum_t = ctx.enter_context(tc.tile_pool(name="psum_t", bufs=4, space="PSUM"))
        psum_o = ctx.enter_context(tc.tile_pool(name="psum_o", bufs=2, space="PSUM"))

        # indices first: the staging copies (and thus the gather pipeline)
        # depend on them
        idx_all = consts.tile([P, TILES * IDXW], mybir.dt.int32)
        nc.sync.dma_start(
            out=idx_all[:].rearrange("p (t w) -> p t w", t=TILES),
            in_=gidx.rearrange("(t p) w -> p t w", t=TILES),
        )

        identity = consts.tile([P, P], mybir.dt.float32)
        nc.sync.dma_start(out=identity[:], in_=ident[:])

        # W^T chunks: wt dram rows (k p) -> sbuf [p, (k e)]
        wt_sb = consts.tile([P, KCHUNKS * E], mybir.dt.float32)
        nc.sync.dma_start(
            out=wt_sb[:].rearrange("p (k e) -> p k e", k=KCHUNKS),
            in_=wt.rearrange("(k p) e -> p k e", k=KCHUNKS),
        )

        out_sb = consts.tile([E, BC], mybir.dt.float32)

        # Prologue: stage every index column into its own contiguous [P,1]
        # tile. The HW descriptor generator only reads the offset AP
        # correctly when it starts at the tile base (offset 0), and doing
        # all copies up front keeps them off the per-tile critical path
        # (the DVE queue is in-order; interleaving them with the reduces
        # would stall the gather pipeline).
        stages = []
        iview = idx_all[:].rearrange("p (t w) -> p t w", t=TILES)
        for t in range(TILES):
            row = []
            for j in range(J):
                st = stpool.tile(
                    [P, 1], mybir.dt.int32, tag=f"st{t}_{j}", name=f"st{t}_{j}"
                )
                nc.vector.tensor_copy(out=st[:], in_=iview[:, t, j : j + 1])
                row.append(st)
            stages.append(row)

        # Process tiles in groups of 4: the transposed chunks of 4 tiles are
        # packed into [P, 512] rhs tiles so each W-chunk LoadStationary is
        # amortized over N=512 (fp32 matmuls are 4-pass; halving PE overhead
        # keeps it off the critical path).
        GRP = 4
        for gi in range(TILES // GRP):
            cts = [
                ctpool.tile(
                    [P, GRP * P], mybir.dt.float32, tag=f"ct{k}", bufs=2,
                    name=f"ct{gi}_{k}",
                )
                for k in range(KCHUNKS)
            ]
            for ti in range(GRP):
                t = gi * GRP + ti
                # one single-index gather per sample into its own whole tile:
                # the multi-index form is mis-ordered and pathologically slow
                # on HW, and indirect-DMA writes to nonzero SBUF offsets fault
                # the exec unit — every gather dest must be a tile base.
                gs = []
                for j in range(J):
                    gj = gpool.tile(
                        [P, F], mybir.dt.float32, tag=f"g{j}", bufs=3,
                        name=f"g{t}_{j}",
                    )
                    nc.gpsimd.indirect_dma_start(
                        out=gj[:],
                        out_offset=None,
                        in_=features[:],
                        in_offset=bass.IndirectOffsetOnAxis(
                            ap=stages[t][j][:], axis=0
                        ),
                    )
                    gs.append(gj)

                # neighbor sum: chained adds
                nsum = spool.tile([P, F], mybir.dt.float32)
                nc.vector.tensor_add(out=nsum[:], in0=gs[1][:], in1=gs[2][:])
                for j in range(3, J):
                    nc.vector.tensor_add(out=nsum[:], in0=nsum[:], in1=gs[j][:])

                for k in range(KCHUNKS):
                    if k < 4:
                        src = gs[0][:, k * P : (k + 1) * P]
                    else:
                        src = nsum[:, (k - 4) * P : (k - 3) * P]
                    pt = psum_t.tile([P, P], mybir.dt.float32)
                    nc.tensor.transpose(out=pt[:], in_=src, identity=identity[:])
                    nc.scalar.copy(out=cts[k][:, ti * P : (ti + 1) * P], in_=pt[:])

            po = psum_o.tile([E, GRP * P], mybir.dt.float32)
            for k in range(KCHUNKS):
                nc.tensor.matmul(
                    out=po[:],
                    lhsT=wt_sb[:, k * E : (k + 1) * E],
                    rhs=cts[k][:],
                    start=(k == 0),
                    stop=(k == KCHUNKS - 1),
                )

            nc.scalar.activation(
                out=out_sb[:, gi * GRP * P : (gi + 1) * GRP * P],
                in_=po[:],
                func=mybir.ActivationFunctionType.Relu,
            )

        nc.sync.dma_start(out=out[:], in_=out_sb[:])

    nc.compile()
    return nc


def _get_nc():
    if "nc" not in _CACHE:
        _CACHE["nc"] = build_nc()
    return _CACHE["nc"]


def make_in_maps(nodes, neigh_idx, features, weight):
    nodes = np.asarray(nodes, dtype=np.int32)
    neigh_idx = np.asarray(neigh_idx, dtype=np.int32)
    features = np.ascontiguousarray(np.asarray(features, dtype=np.float32))
    weight = np.asarray(weight, dtype=np.float32)

    gidx = np.zeros((B, IDXW), dtype=np.int32)
    gidx[:, 0] = nodes
    gidx[:, 1 : J] = neigh_idx

    w = weight.copy()
    w[:, F:] *= 1.0 / S
    wt = np.ascontiguousarray(w.T)  # [2F, E]
    ident = np.eye(P, dtype=np.float32)

    return [
        {
            "gidx": np.ascontiguousarray(gidx[c * BC : (c + 1) * BC]),
            "features": features,
            "wt": wt,
            "ident": ident,
        }
        for c in range(NCORES)
    ]


def run(nodes, neigh_idx, features, weight, trace=False):
    nc = _get_nc()
    in_maps = make_in_maps(nodes, neigh_idx, features, weight)
    res = run_bass_kernel_spmd(nc, in_maps, list(range(NCORES)), trace=trace)
    full = np.concatenate([res.results[c]["out"] for c in range(NCORES)], axis=1)
    return full, res


def kernel(nodes, neigh_idx, features, weight):
    full, _ = run(nodes, neigh_idx, features, weight, trace=False)
    return full



# revision 7
# speedup vs baseline: 1.7476x; 1.0445x over previous
"""GraphSAGE-style mean-aggregator encoder on Trainium2, 8-core SPMD.

Computation (per the reference):
    neigh = features[neigh_idx].mean(1)         # [B, F]
    self_ = features[nodes]                     # [B, F]
    out   = relu(W @ concat(self_, neigh).T)    # [E, B]

Sharding: data-parallel over the node batch B=16384 -> 2048 nodes/core.
features + (pre-transposed, pre-scaled) weight replicated per core.

Per-core kernel, per 128-node tile (16 tiles):
  - 11 single-index indirect DMAs (one per sample) gather 128 rows each
    into whole [128, 512] tiles. HW constraints found empirically: the
    multi-index offset-AP form is mis-ordered and ~70x slower, the offset
    AP must start at a tile base, and indirect-DMA writes to nonzero SBUF
    offsets fault the exec unit. Gathers pipeline at ~1.4us per 256KB
    (Q7 SWDGE descriptor-emission bound, ~181 GB/s/core).
  - neighbor mean via chained DVE adds (1/10 pre-folded into W's
    neighbor half)
  - PE transposes the 8 [128,128] chunks of [self | neigh_sum] via
    identity matmuls, ACT copies PSUM->SBUF; groups of 4 tiles pack rhs
    to N=512 so each W-chunk LoadStationary amortizes (fp32 PE is 4-pass)
  - ACT relu PSUM -> a persistent [128, 2048] output buffer; single 1MB
    store at the end.

Measured on 8xTRN2 (NTFF profile): 282.6us, rel err 4.3e-07.
"""

import numpy as np
import ml_dtypes
from contextlib import ExitStack

import concourse.bass as bass
import concourse.mybir as mybir
import concourse.tile as tile
from concourse import bacc
from concourse.bass_utils import run_bass_kernel_spmd

NCORES = 8
B = 16384
BC = B // NCORES  # 2048 nodes per core
S = 10            # neighbor samples
J = S + 1         # gathered rows per node (self + neighbors)
F = 512           # feature dim
E = 128           # embed dim
NNODES = 200000
P = 128
TILES = BC // P   # 16
IDXW = 16         # padded width of the packed index rows

_CACHE = {}


def build_nc():
    nc = bacc.Bacc(
        "TRN2",
        target_bir_lowering=False,
        debug=False,
        num_devices=NCORES,
    )

    gidx = nc.dram_tensor("gidx", [BC, IDXW], mybir.dt.int32, kind="ExternalInput").ap()
    features = nc.dram_tensor(
        "features", [NNODES, F], mybir.dt.bfloat16, kind="ExternalInput"
    ).ap()
    # host-preprocessed: W^T with the neighbor half pre-scaled by 1/S -> [2F, E]
    wt = nc.dram_tensor("wt", [2 * F, E], mybir.dt.bfloat16, kind="ExternalInput").ap()
    ident = nc.dram_tensor("ident", [P, P], mybir.dt.bfloat16, kind="ExternalInput").ap()
    out = nc.dram_tensor("out", [E, BC], mybir.dt.float32, kind="ExternalOutput").ap()

    KCHUNKS = 2 * F // P  # 8

    with tile.TileContext(nc) as tc, ExitStack() as ctx:
        consts = ctx.enter_context(tc.tile_pool(name="consts", bufs=1))
        stpool = ctx.enter_context(tc.tile_pool(name="stpool", bufs=1))
        gpool = ctx.enter_context(tc.tile_pool(name="gpool", bufs=4))
        spool = ctx.enter_context(tc.tile_pool(name="spool", bufs=3))
        ctpool = ctx.enter_context(tc.tile_pool(name="ctpool", bufs=12))
        psum_t = ctx.enter_context(tc.tile_pool(name="psum_t", bufs=4, space="PSUM"))
        psum_o = ctx.enter_context(tc.tile_pool(name="psum_o", bufs=2, space="PSUM"))

        # indices first: the staging copies (and thus the gather pipeline)
        # depend on them
        idx_all = consts.tile([P, TILES * IDXW], mybir.dt.int32)
        nc.sync.dma_start(
            out=idx_all[:].rearrange("p (t w) -> p t w", t=TILES),
            in_=gidx.rearrange("(t p) w -> p t w", t=TILES),
        )

        identity = consts.tile([P, P], mybir.dt.bfloat16)
        nc.sync.dma_start(out=identity[:], in_=ident[:])

        # W^T chunks: wt dram rows (k p) -> sbuf [p, (k e)]
        wt_sb = consts.tile([P, KCHUNKS * E], mybir.dt.bfloat16)
        nc.sync.dma_start(
            out=wt_sb[:].rearrange("p (k e) -> p k e", k=KCHUNKS),
            in_=wt.rearrange("(k p) e -> p k e", k=KCHUNKS),
        )

        out_sb = consts.tile([E, BC], mybir.dt.float32)

        # Prologue: stage every index column into its own contiguous [P,1]
        # tile. The HW descriptor generator only reads the offset AP
        # correctly when it starts at the tile base (offset 0), and doing
        # all copies up front keeps them off the per-tile critical path
        # (the DVE queue is in-order; interleaving them with the reduces
        # would stall the gather pipeline).
        stages = []
        iview = idx_all[:].rearrange("p (t w) -> p t w", t=TILES)
        for t in range(TILES):
            row = []
            for j in range(J):
                st = stpool.tile(
                    [P, 1], mybir.dt.int32, tag=f"st{t}_{j}", name=f"st{t}_{j}"
                )
                nc.vector.tensor_copy(out=st[:], in_=iview[:, t, j : j + 1])
                row.append(st)
            stages.append(row)

        # Process tiles in groups of 4: the transposed chunks of 4 tiles are
        # packed into [P, 512] rhs tiles so each W-chunk LoadStationary is
        # amortized over N=512 (fp32 matmuls are 4-pass; halving PE overhead
        # keeps it off the critical path).
        GRP = 4
        for gi in range(TILES // GRP):
            cts = [
                ctpool.tile(
                    [P, GRP * P], mybir.dt.bfloat16, tag=f"ct{k}", bufs=2,
                    name=f"ct{gi}_{k}",
                )
                for k in range(KCHUNKS)
            ]
            for ti in range(GRP):
                t = gi * GRP + ti
                # one single-index gather per sample into its own whole tile:
                # the multi-index form is mis-ordered and pathologically slow
                # on HW, and indirect-DMA writes to nonzero SBUF offsets fault
                # the exec unit — every gather dest must be a tile base.
                gs = []
                for j in range(J):
                    gj = gpool.tile(
                        [P, F], mybir.dt.bfloat16, tag=f"g{j}", bufs=4,
                        name=f"g{t}_{j}",
                    )
                    nc.gpsimd.indirect_dma_start(
                        out=gj[:],
                        out_offset=None,
                        in_=features[:],
                        in_offset=bass.IndirectOffsetOnAxis(
                            ap=stages[t][j][:], axis=0
                        ),
                    )
                    gs.append(gj)

                # neighbor sum: chained adds
                nsum = spool.tile([P, F], mybir.dt.bfloat16)
                nc.vector.tensor_add(out=nsum[:], in0=gs[1][:], in1=gs[2][:])
                for j in range(3, J):
                    nc.vector.tensor_add(out=nsum[:], in0=nsum[:], in1=gs[j][:])

                for k in range(KCHUNKS):
                    if k < 4:
                        src = gs[0][:, k * P : (k + 1) * P]
                    else:
                        src = nsum[:, (k - 4) * P : (k - 3) * P]
                    pt = psum_t.tile([P, P], mybir.dt.bfloat16)
                    nc.tensor.transpose(out=pt[:], in_=src, identity=identity[:])
                    nc.scalar.copy(out=cts[k][:, ti * P : (ti + 1) * P], in_=pt[:])

            po = psum_o.tile([E, GRP * P], mybir.dt.float32)
            for k in range(KCHUNKS):
                nc.tensor.matmul(
                    out=po[:],
                    lhsT=wt_sb[:, k * E : (k + 1) * E],
                    rhs=cts[k][:],
                    start=(k == 0),
                    stop=(k == KCHUNKS - 1),
                )

            nc.scalar.activation(
                out=out_sb[:, gi * GRP * P : (gi + 1) * GRP * P],
                in_=po[:],
                func=mybir.ActivationFunctionType.Relu,
            )

        nc.sync.dma_start(out=out[:], in_=out_sb[:])

    nc.compile()
    return nc


def _get_nc():
    if "nc" not in _CACHE:
        _CACHE["nc"] = build_nc()
    return _CACHE["nc"]


def make_in_maps(nodes, neigh_idx, features, weight):
    nodes = np.asarray(nodes, dtype=np.int32)
    neigh_idx = np.asarray(neigh_idx, dtype=np.int32)
    features = np.ascontiguousarray(np.asarray(features, dtype=np.float32).astype(ml_dtypes.bfloat16))
    weight = np.asarray(weight, dtype=np.float32)

    gidx = np.zeros((B, IDXW), dtype=np.int32)
    gidx[:, 0] = nodes
    gidx[:, 1 : J] = neigh_idx

    w = weight.copy()
    w[:, F:] *= 1.0 / S
    wt = np.ascontiguousarray(w.T.astype(ml_dtypes.bfloat16))  # [2F, E]
    ident = np.eye(P, dtype=np.float32).astype(ml_dtypes.bfloat16)

    return [
        {
            "gidx": np.ascontiguousarray(gidx[c * BC : (c + 1) * BC]),
            "features": features,
            "wt": wt,
            "ident": ident,
        }
        for c in range(NCORES)
    ]


def run(nodes, neigh_idx, features, weight, trace=False):
    nc = _get_nc()
    in_maps = make_in_maps(nodes, neigh_idx, features, weight)
    res = run_bass_kernel_spmd(nc, in_maps, list(range(NCORES)), trace=trace)
    full = np.concatenate([res.results[c]["out"] for c in range(NCORES)], axis=1)
    return full, res


def kernel(nodes, neigh_idx, features, weight):
    full, _ = run(nodes, neigh_idx, features, weight, trace=False)
    return full



# revision 8
# speedup vs baseline: 1.7533x; 1.0033x over previous
"""GraphSAGE-style mean-aggregator encoder on Trainium2, 8-core SPMD.

Computation (per the reference):
    neigh = features[neigh_idx].mean(1)         # [B, F]
    self_ = features[nodes]                     # [B, F]
    out   = relu(W @ concat(self_, neigh).T)    # [E, B]

Sharding: data-parallel over the node batch B=16384 -> 2048 nodes/core.
features + (pre-transposed, pre-scaled) weight replicated per core.

Per-core kernel, per 128-node tile (16 tiles):
  - 11 single-index indirect DMAs (one per sample) gather 128 rows each
    into whole [128, 512] tiles. HW constraints found empirically: the
    multi-index offset-AP form is mis-ordered and ~70x slower, the offset
    AP must start at a tile base, and indirect-DMA writes to nonzero SBUF
    offsets fault the exec unit. Gathers pipeline at ~1.4us per 256KB
    (Q7 SWDGE descriptor-emission bound, ~181 GB/s/core).
  - neighbor mean via chained DVE adds (1/10 pre-folded into W's
    neighbor half)
  - PE transposes the 8 [128,128] chunks of [self | neigh_sum] via
    identity matmuls, ACT copies PSUM->SBUF; groups of 4 tiles pack rhs
    to N=512 so each W-chunk LoadStationary amortizes (fp32 PE is 4-pass)
  - ACT relu PSUM -> a persistent [128, 2048] output buffer; single 1MB
    store at the end.

Measured on 8xTRN2 (NTFF profile): 282.6us, rel err 4.3e-07.
"""

import numpy as np
import ml_dtypes
from contextlib import ExitStack

import concourse.bass as bass
import concourse.mybir as mybir
import concourse.tile as tile
from concourse import bacc
from concourse.bass_utils import run_bass_kernel_spmd

NCORES = 8
B = 16384
BC = B // NCORES  # 2048 nodes per core
S = 10            # neighbor samples
J = S + 1         # gathered rows per node (self + neighbors)
F = 512           # feature dim
E = 128           # embed dim
NNODES = 200000
P = 128
TILES = BC // P   # 16
IDXW = 16         # padded width of the packed index rows

_CACHE = {}


def build_nc():
    nc = bacc.Bacc(
        "TRN2",
        target_bir_lowering=False,
        debug=False,
        num_devices=NCORES,
    )

    gidx = nc.dram_tensor("gidx", [BC, IDXW], mybir.dt.int32, kind="ExternalInput").ap()
    features = nc.dram_tensor(
        "features", [NNODES, F], mybir.dt.bfloat16, kind="ExternalInput"
    ).ap()
    # host-preprocessed: W^T with the neighbor half pre-scaled by 1/S -> [2F, E]
    wt = nc.dram_tensor("wt", [2 * F, E], mybir.dt.bfloat16, kind="ExternalInput").ap()
    ident = nc.dram_tensor("ident", [P, P], mybir.dt.bfloat16, kind="ExternalInput").ap()
    out = nc.dram_tensor("out", [E, BC], mybir.dt.float32, kind="ExternalOutput").ap()

    KCHUNKS = 2 * F // P  # 8

    with tile.TileContext(nc) as tc, ExitStack() as ctx:
        consts = ctx.enter_context(tc.tile_pool(name="consts", bufs=1))
        stpool = ctx.enter_context(tc.tile_pool(name="stpool", bufs=1))
        gpool = ctx.enter_context(tc.tile_pool(name="gpool", bufs=4))
        spool = ctx.enter_context(tc.tile_pool(name="spool", bufs=3))
        ctpool = ctx.enter_context(tc.tile_pool(name="ctpool", bufs=12))
        psum_t = ctx.enter_context(tc.tile_pool(name="psum_t", bufs=4, space="PSUM"))
        psum_o = ctx.enter_context(tc.tile_pool(name="psum_o", bufs=2, space="PSUM"))

        # indices first: the staging copies (and thus the gather pipeline)
        # depend on them
        idx_all = consts.tile([P, TILES * IDXW], mybir.dt.int32)
        nc.sync.dma_start(
            out=idx_all[:].rearrange("p (t w) -> p t w", t=TILES),
            in_=gidx.rearrange("(t p) w -> p t w", t=TILES),
        )

        identity = consts.tile([P, P], mybir.dt.bfloat16)
        nc.sync.dma_start(out=identity[:], in_=ident[:])

        # W^T chunks: wt dram rows (k p) -> sbuf [p, (k e)]
        wt_sb = consts.tile([P, KCHUNKS * E], mybir.dt.bfloat16)
        nc.sync.dma_start(
            out=wt_sb[:].rearrange("p (k e) -> p k e", k=KCHUNKS),
            in_=wt.rearrange("(k p) e -> p k e", k=KCHUNKS),
        )

        out_sb = consts.tile([E, BC], mybir.dt.float32)

        # Prologue: stage every index column into its own contiguous [P,1]
        # tile. The HW descriptor generator only reads the offset AP
        # correctly when it starts at the tile base (offset 0), and doing
        # all copies up front keeps them off the per-tile critical path
        # (the DVE queue is in-order; interleaving them with the reduces
        # would stall the gather pipeline).
        stages = []
        iview = idx_all[:].rearrange("p (t w) -> p t w", t=TILES)
        for t in range(TILES):
            row = []
            for j in range(J):
                st = stpool.tile(
                    [P, 1], mybir.dt.int32, tag=f"st{t}_{j}", name=f"st{t}_{j}"
                )
                nc.vector.tensor_copy(out=st[:], in_=iview[:, t, j : j + 1])
                row.append(st)
            stages.append(row)

        # Process tiles in groups of 4: the transposed chunks of 4 tiles are
        # packed into [P, 512] rhs tiles so each W-chunk LoadStationary is
        # amortized over N=512 (fp32 matmuls are 4-pass; halving PE overhead
        # keeps it off the critical path).
        GRP = 4
        for gi in range(TILES // GRP):
            cts = [
                ctpool.tile(
                    [P, GRP * P], mybir.dt.bfloat16, tag=f"ct{k}", bufs=2,
                    name=f"ct{gi}_{k}",
                )
                for k in range(KCHUNKS)
            ]
            for ti in range(GRP):
                t = gi * GRP + ti
                # one single-index gather per sample into its own whole tile:
                # the multi-index form is mis-ordered and pathologically slow
                # on HW, and indirect-DMA writes to nonzero SBUF offsets fault
                # the exec unit — every gather dest must be a tile base.
                gs = []
                for j in range(J):
                    gj = gpool.tile(
                        [P, F], mybir.dt.bfloat16, tag=f"g{j}", bufs=4,
                        name=f"g{t}_{j}",
                    )
                    nc.gpsimd.indirect_dma_start(
                        out=gj[:],
                        out_offset=None,
                        in_=features[:],
                        in_offset=bass.IndirectOffsetOnAxis(
                            ap=stages[t][j][:], axis=0
                        ),
                    )
                    gs.append(gj)

                # neighbor sum: chained adds
                nsum = spool.tile([P, F], mybir.dt.bfloat16)
                nc.vector.tensor_add(out=nsum[:], in0=gs[1][:], in1=gs[2][:])
                for j in range(3, J):
                    nc.vector.tensor_add(out=nsum[:], in0=nsum[:], in1=gs[j][:])

                for k in range(KCHUNKS):
                    if k < 4:
                        src = gs[0][:, k * P : (k + 1) * P]
                    else:
                        src = nsum[:, (k - 4) * P : (k - 3) * P]
                    pt = psum_t.tile([P, P], mybir.dt.bfloat16)
                    nc.tensor.transpose(out=pt[:], in_=src, identity=identity[:])
                    nc.scalar.copy(out=cts[k][:, ti * P : (ti + 1) * P], in_=pt[:])

            po = psum_o.tile([E, GRP * P], mybir.dt.float32)
            for k in range(KCHUNKS):
                nc.tensor.matmul(
                    out=po[:],
                    lhsT=wt_sb[:, k * E : (k + 1) * E],
                    rhs=cts[k][:],
                    start=(k == 0),
                    stop=(k == KCHUNKS - 1),
                )

            nc.scalar.activation(
                out=out_sb[:, gi * GRP * P : (gi + 1) * GRP * P],
                in_=po[:],
                func=mybir.ActivationFunctionType.Relu,
            )
            nc.sync.dma_start(
                out=out[:, gi * GRP * P : (gi + 1) * GRP * P],
                in_=out_sb[:, gi * GRP * P : (gi + 1) * GRP * P],
            )

    nc.compile()
    return nc


def _get_nc():
    if "nc" not in _CACHE:
        _CACHE["nc"] = build_nc()
    return _CACHE["nc"]


def make_in_maps(nodes, neigh_idx, features, weight):
    nodes = np.asarray(nodes, dtype=np.int32)
    neigh_idx = np.asarray(neigh_idx, dtype=np.int32)
    features = np.ascontiguousarray(np.asarray(features, dtype=np.float32).astype(ml_dtypes.bfloat16))
    weight = np.asarray(weight, dtype=np.float32)

    gidx = np.zeros((B, IDXW), dtype=np.int32)
    gidx[:, 0] = nodes
    gidx[:, 1 : J] = neigh_idx

    w = weight.copy()
    w[:, F:] *= 1.0 / S
    wt = np.ascontiguousarray(w.T.astype(ml_dtypes.bfloat16))  # [2F, E]
    ident = np.eye(P, dtype=np.float32).astype(ml_dtypes.bfloat16)

    return [
        {
            "gidx": np.ascontiguousarray(gidx[c * BC : (c + 1) * BC]),
            "features": features,
            "wt": wt,
            "ident": ident,
        }
        for c in range(NCORES)
    ]


def run(nodes, neigh_idx, features, weight, trace=False):
    nc = _get_nc()
    in_maps = make_in_maps(nodes, neigh_idx, features, weight)
    res = run_bass_kernel_spmd(nc, in_maps, list(range(NCORES)), trace=trace)
    full = np.concatenate([res.results[c]["out"] for c in range(NCORES)], axis=1)
    return full, res


def kernel(nodes, neigh_idx, features, weight):
    full, _ = run(nodes, neigh_idx, features, weight, trace=False)
    return full



# revision 9
# speedup vs baseline: 1.7610x; 1.0044x over previous
"""GraphSAGE-style mean-aggregator encoder on Trainium2, 8-core SPMD.

Computation (per the reference):
    neigh = features[neigh_idx].mean(1)         # [B, F]
    self_ = features[nodes]                     # [B, F]
    out   = relu(W @ concat(self_, neigh).T)    # [E, B]

Sharding: data-parallel over the node batch B=16384 -> 2048 nodes/core.
features + (pre-transposed, pre-scaled) weight replicated per core.

Per-core kernel, per 128-node tile (16 tiles):
  - 11 single-index indirect DMAs (one per sample) gather 128 rows each
    into whole [128, 512] tiles. HW constraints found empirically: the
    multi-index offset-AP form is mis-ordered and ~70x slower, the offset
    AP must start at a tile base, and indirect-DMA writes to nonzero SBUF
    offsets fault the exec unit. Gathers pipeline at ~1.4us per 256KB
    (Q7 SWDGE descriptor-emission bound, ~181 GB/s/core).
  - neighbor mean via chained DVE adds (1/10 pre-folded into W's
    neighbor half)
  - PE transposes the 8 [128,128] chunks of [self | neigh_sum] via
    identity matmuls, ACT copies PSUM->SBUF; groups of 4 tiles pack rhs
    to N=512 so each W-chunk LoadStationary amortizes (fp32 PE is 4-pass)
  - ACT relu PSUM -> a persistent [128, 2048] output buffer; single 1MB
    store at the end.

Measured on 8xTRN2 (NTFF profile): 282.6us, rel err 4.3e-07.
"""

import numpy as np
import ml_dtypes
from contextlib import ExitStack

import concourse.bass as bass
import concourse.mybir as mybir
import concourse.tile as tile
from concourse import bacc
from concourse.bass_utils import run_bass_kernel_spmd

NCORES = 8
B = 16384
BC = B // NCORES  # 2048 nodes per core
S = 10            # neighbor samples
J = S + 1         # gathered rows per node (self + neighbors)
F = 512           # feature dim
E = 128           # embed dim
NNODES = 200000
P = 128
TILES = BC // P   # 16
IDXW = 16         # padded width of the packed index rows

_CACHE = {}


def build_nc():
    nc = bacc.Bacc(
        "TRN2",
        target_bir_lowering=False,
        debug=False,
        num_devices=NCORES,
    )

    gidx = nc.dram_tensor("gidx", [BC, IDXW], mybir.dt.int32, kind="ExternalInput").ap()
    features = nc.dram_tensor(
        "features", [NNODES, F], mybir.dt.bfloat16, kind="ExternalInput"
    ).ap()
    # host-preprocessed: W^T with the neighbor half pre-scaled by 1/S -> [2F, E]
    wt = nc.dram_tensor("wt", [2 * F, E], mybir.dt.bfloat16, kind="ExternalInput").ap()
    ident = nc.dram_tensor("ident", [P, P], mybir.dt.bfloat16, kind="ExternalInput").ap()
    out = nc.dram_tensor("out", [E, BC], mybir.dt.float32, kind="ExternalOutput").ap()

    KCHUNKS = 2 * F // P  # 8

    with tile.TileContext(nc) as tc, ExitStack() as ctx:
        consts = ctx.enter_context(tc.tile_pool(name="consts", bufs=1))
        stpool = ctx.enter_context(tc.tile_pool(name="stpool", bufs=1))
        gpool = ctx.enter_context(tc.tile_pool(name="gpool", bufs=4))
        spool = ctx.enter_context(tc.tile_pool(name="spool", bufs=3))
        ctpool = ctx.enter_context(tc.tile_pool(name="ctpool", bufs=12))
        psum_t = ctx.enter_context(tc.tile_pool(name="psum_t", bufs=4, space="PSUM"))
        psum_o = ctx.enter_context(tc.tile_pool(name="psum_o", bufs=2, space="PSUM"))

        # indices first: the staging copies (and thus the gather pipeline)
        # depend on them
        idx_all = consts.tile([P, TILES * IDXW], mybir.dt.int32)
        giv = gidx.rearrange("(t p) w -> p t w", t=TILES)
        iav = idx_all[:].rearrange("p (t w) -> p t w", t=TILES)
        nc.sync.dma_start(out=iav[:, 0:1, :], in_=giv[:, 0:1, :])
        nc.sync.dma_start(out=iav[:, 1:, :], in_=giv[:, 1:, :])

        identity = consts.tile([P, P], mybir.dt.bfloat16)
        nc.sync.dma_start(out=identity[:], in_=ident[:])

        # W^T chunks: wt dram rows (k p) -> sbuf [p, (k e)]
        wt_sb = consts.tile([P, KCHUNKS * E], mybir.dt.bfloat16)
        nc.sync.dma_start(
            out=wt_sb[:].rearrange("p (k e) -> p k e", k=KCHUNKS),
            in_=wt.rearrange("(k p) e -> p k e", k=KCHUNKS),
        )

        out_sb = consts.tile([E, BC], mybir.dt.float32)

        # Prologue: stage every index column into its own contiguous [P,1]
        # tile. The HW descriptor generator only reads the offset AP
        # correctly when it starts at the tile base (offset 0), and doing
        # all copies up front keeps them off the per-tile critical path
        # (the DVE queue is in-order; interleaving them with the reduces
        # would stall the gather pipeline).
        stages = []
        iview = idx_all[:].rearrange("p (t w) -> p t w", t=TILES)
        for t in range(TILES):
            row = []
            for j in range(J):
                st = stpool.tile(
                    [P, 1], mybir.dt.int32, tag=f"st{t}_{j}", name=f"st{t}_{j}"
                )
                nc.vector.tensor_copy(out=st[:], in_=iview[:, t, j : j + 1])
                row.append(st)
            stages.append(row)

        # Process tiles in groups of 4: the transposed chunks of 4 tiles are
        # packed into [P, 512] rhs tiles so each W-chunk LoadStationary is
        # amortized over N=512 (fp32 matmuls are 4-pass; halving PE overhead
        # keeps it off the critical path).
        GRP = 4
        for gi in range(TILES // GRP):
            cts = [
                ctpool.tile(
                    [P, GRP * P], mybir.dt.bfloat16, tag=f"ct{k}", bufs=2,
                    name=f"ct{gi}_{k}",
                )
                for k in range(KCHUNKS)
            ]
            for ti in range(GRP):
                t = gi * GRP + ti
                # one single-index gather per sample into its own whole tile:
                # the multi-index form is mis-ordered and pathologically slow
                # on HW, and indirect-DMA writes to nonzero SBUF offsets fault
                # the exec unit — every gather dest must be a tile base.
                gs = []
                for j in range(J):
                    gj = gpool.tile(
                        [P, F], mybir.dt.bfloat16, tag=f"g{j}", bufs=4,
                        name=f"g{t}_{j}",
                    )
                    nc.gpsimd.indirect_dma_start(
                        out=gj[:],
                        out_offset=None,
                        in_=features[:],
                        in_offset=bass.IndirectOffsetOnAxis(
                            ap=stages[t][j][:], axis=0
                        ),
                    )
                    gs.append(gj)

                # neighbor sum: chained adds
                # tree-shaped neighbor sum: depth 4 instead of a 9-deep chain
                pa = [
                    spool.tile([P, F], mybir.dt.bfloat16, tag=f"pa{i}", bufs=2,
                               name=f"pa{t}_{i}")
                    for i in range(5)
                ]
                for i in range(5):
                    nc.vector.tensor_add(
                        out=pa[i][:], in0=gs[1 + 2 * i][:], in1=gs[2 + 2 * i][:]
                    )
                nc.vector.tensor_add(out=pa[0][:], in0=pa[0][:], in1=pa[1][:])
                nc.vector.tensor_add(out=pa[2][:], in0=pa[2][:], in1=pa[3][:])
                nc.vector.tensor_add(out=pa[0][:], in0=pa[0][:], in1=pa[2][:])
                nsum = pa[0]
                nc.vector.tensor_add(out=nsum[:], in0=nsum[:], in1=pa[4][:])

                for k in range(KCHUNKS):
                    if k < 4:
                        src = gs[0][:, k * P : (k + 1) * P]
                    else:
                        src = nsum[:, (k - 4) * P : (k - 3) * P]
                    pt = psum_t.tile([P, P], mybir.dt.bfloat16)
                    nc.tensor.transpose(out=pt[:], in_=src, identity=identity[:])
                    nc.scalar.copy(out=cts[k][:, ti * P : (ti + 1) * P], in_=pt[:])

            po = psum_o.tile([E, GRP * P], mybir.dt.float32)
            for k in range(KCHUNKS):
                nc.tensor.matmul(
                    out=po[:],
                    lhsT=wt_sb[:, k * E : (k + 1) * E],
                    rhs=cts[k][:],
                    start=(k == 0),
                    stop=(k == KCHUNKS - 1),
                )

            nc.scalar.activation(
                out=out_sb[:, gi * GRP * P : (gi + 1) * GRP * P],
                in_=po[:],
                func=mybir.ActivationFunctionType.Relu,
            )
            nc.sync.dma_start(
                out=out[:, gi * GRP * P : (gi + 1) * GRP * P],
                in_=out_sb[:, gi * GRP * P : (gi + 1) * GRP * P],
            )

    nc.compile()
    return nc


def _get_nc():
    if "nc" not in _CACHE:
        _CACHE["nc"] = build_nc()
    return _CACHE["nc"]


def make_in_maps(nodes, neigh_idx, features, weight):
    nodes = np.asarray(nodes, dtype=np.int32)
    neigh_idx = np.asarray(neigh_idx, dtype=np.int32)
    features = np.ascontiguousarray(np.asarray(features, dtype=np.float32).astype(ml_dtypes.bfloat16))
    weight = np.asarray(weight, dtype=np.float32)

    gidx = np.zeros((B, IDXW), dtype=np.int32)
    gidx[:, 0] = nodes
    gidx[:, 1 : J] = neigh_idx

    w = weight.copy()
    w[:, F:] *= 1.0 / S
    wt = np.ascontiguousarray(w.T.astype(ml_dtypes.bfloat16))  # [2F, E]
    ident = np.eye(P, dtype=np.float32).astype(ml_dtypes.bfloat16)

    return [
        {
            "gidx": np.ascontiguousarray(gidx[c * BC : (c + 1) * BC]),
            "features": features,
            "wt": wt,
            "ident": ident,
        }
        for c in range(NCORES)
    ]


def run(nodes, neigh_idx, features, weight, trace=False):
    nc = _get_nc()
    in_maps = make_in_maps(nodes, neigh_idx, features, weight)
    res = run_bass_kernel_spmd(nc, in_maps, list(range(NCORES)), trace=trace)
    full = np.concatenate([res.results[c]["out"] for c in range(NCORES)], axis=1)
    return full, res


def kernel(nodes, neigh_idx, features, weight):
    full, _ = run(nodes, neigh_idx, features, weight, trace=False)
    return full



# revision 13
# speedup vs baseline: 1.7704x; 1.0053x over previous
"""GraphSAGE-style mean-aggregator encoder on Trainium2, 8-core SPMD.

Computation (per the reference):
    neigh = features[neigh_idx].mean(1)         # [B, F]
    self_ = features[nodes]                     # [B, F]
    out   = relu(W @ concat(self_, neigh).T)    # [E, B]

Sharding: data-parallel over the node batch B=16384 -> 2048 nodes/core.
features + (pre-transposed, pre-scaled) weight replicated per core.

Per-core kernel, per 128-node tile (16 tiles):
  - 11 single-index indirect DMAs (one per sample) gather 128 rows each
    into whole [128, 512] tiles. HW constraints found empirically: the
    multi-index offset-AP form is mis-ordered and ~70x slower, the offset
    AP must start at a tile base, and indirect-DMA writes to nonzero SBUF
    offsets fault the exec unit. Gathers pipeline at ~1.4us per 256KB
    (Q7 SWDGE descriptor-emission bound, ~181 GB/s/core).
  - neighbor mean via chained DVE adds (1/10 pre-folded into W's
    neighbor half)
  - PE transposes the 8 [128,128] chunks of [self | neigh_sum] via
    identity matmuls, ACT copies PSUM->SBUF; groups of 4 tiles pack rhs
    to N=512 so each W-chunk LoadStationary amortizes (fp32 PE is 4-pass)
  - ACT relu PSUM -> a persistent [128, 2048] output buffer; single 1MB
    store at the end.

Measured on 8xTRN2 (NTFF profile): 282.6us, rel err 4.3e-07.
"""

import numpy as np
import ml_dtypes
from contextlib import ExitStack

import concourse.bass as bass
import concourse.mybir as mybir
import concourse.tile as tile
from concourse import bacc
from concourse.bass_utils import run_bass_kernel_spmd

NCORES = 8
B = 16384
BC = B // NCORES  # 2048 nodes per core
S = 10            # neighbor samples
J = S + 1         # gathered rows per node (self + neighbors)
F = 512           # feature dim
E = 128           # embed dim
NNODES = 200000
P = 128
TILES = BC // P   # 16
IDXW = 16         # padded width of the packed index rows

_CACHE = {}


def build_nc():
    nc = bacc.Bacc(
        "TRN2",
        target_bir_lowering=False,
        debug=False,
        num_devices=NCORES,
    )

    gidx = nc.dram_tensor("gidx", [BC, IDXW], mybir.dt.int32, kind="ExternalInput").ap()
    features = nc.dram_tensor(
        "features", [NNODES, F], mybir.dt.bfloat16, kind="ExternalInput"
    ).ap()
    # host-preprocessed: W^T with the neighbor half pre-scaled by 1/S -> [2F, E]
    wt = nc.dram_tensor("wt", [2 * F, E], mybir.dt.bfloat16, kind="ExternalInput").ap()
    ident = nc.dram_tensor("ident", [P, P], mybir.dt.bfloat16, kind="ExternalInput").ap()
    out = nc.dram_tensor("out", [E, BC], mybir.dt.float32, kind="ExternalOutput").ap()

    KCHUNKS = 2 * F // P  # 8

    with tile.TileContext(nc) as tc, ExitStack() as ctx:
        consts = ctx.enter_context(tc.tile_pool(name="consts", bufs=1))
        stpool = ctx.enter_context(tc.tile_pool(name="stpool", bufs=1))
        gpool = ctx.enter_context(tc.tile_pool(name="gpool", bufs=4))
        spool = ctx.enter_context(tc.tile_pool(name="spool", bufs=3))
        ctpool = ctx.enter_context(tc.tile_pool(name="ctpool", bufs=12))
        psum_t = ctx.enter_context(tc.tile_pool(name="psum_t", bufs=4, space="PSUM"))
        psum_o = ctx.enter_context(tc.tile_pool(name="psum_o", bufs=2, space="PSUM"))

        # indices first: the staging copies (and thus the gather pipeline)
        # depend on them
        idx_all = consts.tile([P, TILES * IDXW], mybir.dt.int32)
        giv = gidx.rearrange("(t p) w -> p t w", t=TILES)
        iav = idx_all[:].rearrange("p (t w) -> p t w", t=TILES)
        nc.sync.dma_start(out=iav[:, 0:1, :], in_=giv[:, 0:1, :])
        nc.sync.dma_start(out=iav[:, 1:, :], in_=giv[:, 1:, :])

        identity = consts.tile([P, P], mybir.dt.bfloat16)
        nc.sync.dma_start(out=identity[:], in_=ident[:])

        # W^T chunks: wt dram rows (k p) -> sbuf [p, (k e)]
        wt_sb = consts.tile([P, KCHUNKS * E], mybir.dt.bfloat16)
        nc.sync.dma_start(
            out=wt_sb[:].rearrange("p (k e) -> p k e", k=KCHUNKS),
            in_=wt.rearrange("(k p) e -> p k e", k=KCHUNKS),
        )

        out_sb = consts.tile([E, BC], mybir.dt.float32)

        # Prologue: stage every index column into its own contiguous [P,1]
        # tile. The HW descriptor generator only reads the offset AP
        # correctly when it starts at the tile base (offset 0), and doing
        # all copies up front keeps them off the per-tile critical path
        # (the DVE queue is in-order; interleaving them with the reduces
        # would stall the gather pipeline).
        stages = []
        iview = idx_all[:].rearrange("p (t w) -> p t w", t=TILES)
        for t in range(TILES):
            row = []
            for j in range(J):
                st = stpool.tile(
                    [P, 1], mybir.dt.int32, tag=f"st{t}_{j}", name=f"st{t}_{j}"
                )
                nc.vector.tensor_copy(out=st[:], in_=iview[:, t, j : j + 1])
                row.append(st)
            stages.append(row)

        # Process tiles in groups of 4: the transposed chunks of 4 tiles are
        # packed into [P, 512] rhs tiles so each W-chunk LoadStationary is
        # amortized over N=512 (fp32 matmuls are 4-pass; halving PE overhead
        # keeps it off the critical path).
        GRP = 4

        def emit_chain(gi, cts, h, halves):
            hw_cols = GRP * P // halves
            po = psum_o.tile(
                [E, hw_cols], mybir.dt.float32, tag=f"po{halves}",
                name=f"po_{gi}_{h}",
            )
            for k in range(KCHUNKS):
                nc.tensor.matmul(
                    out=po[:],
                    lhsT=wt_sb[:, k * E : (k + 1) * E],
                    rhs=cts[k][:, h * hw_cols : (h + 1) * hw_cols],
                    start=(k == 0),
                    stop=(k == KCHUNKS - 1),
                )
            c0 = gi * GRP * P + h * hw_cols
            nc.scalar.activation(
                out=out_sb[:, c0 : c0 + hw_cols],
                in_=po[:],
                func=mybir.ActivationFunctionType.Relu,
            )
            nc.sync.dma_start(
                out=out[:, c0 : c0 + hw_cols], in_=out_sb[:, c0 : c0 + hw_cols]
            )

        for gi in range(TILES // GRP):
            cts = [
                ctpool.tile(
                    [P, GRP * P], mybir.dt.bfloat16, tag=f"ct{k}", bufs=2,
                    name=f"ct{gi}_{k}",
                )
                for k in range(KCHUNKS)
            ]
            for ti in range(GRP):
                t = gi * GRP + ti
                # one single-index gather per sample into its own whole tile:
                # the multi-index form is mis-ordered and pathologically slow
                # on HW, and indirect-DMA writes to nonzero SBUF offsets fault
                # the exec unit — every gather dest must be a tile base.
                gs = []
                for j in range(J):
                    gj = gpool.tile(
                        [P, F], mybir.dt.bfloat16, tag=f"g{j}", bufs=4,
                        name=f"g{t}_{j}",
                    )
                    nc.gpsimd.indirect_dma_start(
                        out=gj[:],
                        out_offset=None,
                        in_=features[:],
                        in_offset=bass.IndirectOffsetOnAxis(
                            ap=stages[t][j][:], axis=0
                        ),
                    )
                    gs.append(gj)

                # neighbor sum: chained adds
                # tree-shaped neighbor sum: depth 4 instead of a 9-deep chain
                pa = [
                    spool.tile([P, F], mybir.dt.bfloat16, tag=f"pa{i}", bufs=2,
                               name=f"pa{t}_{i}")
                    for i in range(5)
                ]
                for i in range(5):
                    nc.vector.tensor_add(
                        out=pa[i][:], in0=gs[1 + 2 * i][:], in1=gs[2 + 2 * i][:]
                    )
                nc.vector.tensor_add(out=pa[0][:], in0=pa[0][:], in1=pa[1][:])
                nc.vector.tensor_add(out=pa[2][:], in0=pa[2][:], in1=pa[3][:])
                nc.vector.tensor_add(out=pa[0][:], in0=pa[0][:], in1=pa[2][:])
                nsum = pa[0]
                nc.vector.tensor_add(out=nsum[:], in0=nsum[:], in1=pa[4][:])

                for k in range(KCHUNKS):
                    if k < 4:
                        src = gs[0][:, k * P : (k + 1) * P]
                    else:
                        src = nsum[:, (k - 4) * P : (k - 3) * P]
                    pt = psum_t.tile([P, P], mybir.dt.bfloat16)
                    nc.tensor.transpose(out=pt[:], in_=src, identity=identity[:])
                    nc.scalar.copy(out=cts[k][:, ti * P : (ti + 1) * P], in_=pt[:])

                last = gi == TILES // GRP - 1
                if last and ti == GRP // 2 - 1:
                    # half-group chain for tiles 12-13 runs while 14-15 gather
                    emit_chain(gi, cts, 0, 2)
                elif last and ti == GRP - 1:
                    emit_chain(gi, cts, 1, 2)
                elif not last and ti == GRP - 1:
                    emit_chain(gi, cts, 0, 1)


    nc.compile()
    return nc


def _get_nc():
    if "nc" not in _CACHE:
        _CACHE["nc"] = build_nc()
    return _CACHE["nc"]


def make_in_maps(nodes, neigh_idx, features, weight):
    nodes = np.asarray(nodes, dtype=np.int32)
    neigh_idx = np.asarray(neigh_idx, dtype=np.int32)
    features = np.ascontiguousarray(np.asarray(features, dtype=np.float32).astype(ml_dtypes.bfloat16))
    weight = np.asarray(weight, dtype=np.float32)

    gidx = np.zeros((B, IDXW), dtype=np.int32)
    gidx[:, 0] = nodes
    gidx[:, 1 : J] = neigh_idx

    w = weight.copy()
    w[:, F:] *= 1.0 / S
    wt = np.ascontiguousarray(w.T.astype(ml_dtypes.bfloat16))  # [2F, E]
    ident = np.eye(P, dtype=np.float32).astype(ml_dtypes.bfloat16)

    return [
        {
            "gidx": np.ascontiguousarray(gidx[c * BC : (c + 1) * BC]),
            "features": features,
            "wt": wt,
            "ident": ident,
        }
        for c in range(NCORES)
    ]


def run(nodes, neigh_idx, features, weight, trace=False):
    nc = _get_nc()
    in_maps = make_in_maps(nodes, neigh_idx, features, weight)
    res = run_bass_kernel_spmd(nc, in_maps, list(range(NCORES)), trace=trace)
    full = np.concatenate([res.results[c]["out"] for c in range(NCORES)], axis=1)
    return full, res


def kernel(nodes, neigh_idx, features, weight):
    full, _ = run(nodes, neigh_idx, features, weight, trace=False)
    return full

